# revision 57
# baseline (speedup 1.0000x reference)
"""BiAffineParser span-classifier kernel for 8 Trainium2 NeuronCores.

Computes logits[b,i,j,n] = gelu(xs_proj[b,i] + xe_proj[b,j] + b1) @ W2 + b2
for the full L x L span grid without materializing the (B,L,L,H) tensor in
HBM.  Sharding: 8 cores = 4 batches x 2 halves of the i axis; each core
produces a (128, 256, 13) output shard.

Per-core dataflow (H=768 split into 6 chunks of 128 partitions):
  PE   : xsT/xeT projections (bf16), then W2 contraction with the gelu tile
         as the stationary operand (bf16) so output lands j-major in PSUM.
  DVE  : broadcast-add xeT[h,j] + (xsT+b1)[h,i] in bf16, plus a clipped-
         cubic gelu approximation on a small slice of each group (chain
         ops interleaved into the broadcast stream) to offload ACT.
  ACT  : exact-erf Gelu on [128, 8192] SBUF tiles (the 1 col/cycle
         bottleneck; ~0.86 ns/col including instruction init).
  DMA  : HWDGE loads (bf16 x/W1); j-major contiguous output stores
         (1664B bursts; host transposes back to i-major).
"""

import os
import sys

if "/opt/trn_rl_repo" not in sys.path:
    sys.path.insert(0, "/opt/trn_rl_repo")

import numpy as np

B = 4
L = 256
H = 768
NH = 6           # 128-partition chunks of H
NL = 13          # num labels
IH = 128         # i rows per core
G = 32           # i-group size for the steady-state pipeline
NGRP = IH // G   # groups per core
GC = NL * G      # psum columns per j-tile per group

# Clipped-cubic gelu approximation evaluated on DVE (bf16, 5 ops at ~2
# cyc/elem) for a slice of the last h-chunk of each i-group, offloading
# the ACT engine (the 1 col/cycle bottleneck):
#   z = clamp(u, +-C);  gelu(u) ~= u * (0.5 + z*(ALPHA - BETA*z^2))
# Fitted on the actual input distribution; end-to-end max rel err ~7e-3
# (tolerance 2e-2).
ALPHA = 0.383016
BETA = 0.040330
CLIP = 1.779227
# Columns of tile (g, c=CHAIN_C) computed by the DVE chain instead of ACT.
# The 5 chain ops are interleaved one-per-chunk into the broadcast-add
# stream so they fill DVE's wait slots instead of stalling the pipeline.
CHAIN_C = 0
DVE_COLS = {}

_CACHE = {}


def _build(repeat=1):
    import concourse.mybir as mybir
    from concourse import bacc
    from concourse.tile import TileContext

    f32 = mybir.dt.float32
    bf16 = mybir.dt.bfloat16
    f32r = mybir.dt.float32r
    GELU = mybir.ActivationFunctionType.Gelu
    ALU = mybir.AluOpType

    nc = bacc.Bacc("TRN2", target_bir_lowering=False)

    xt_d = nc.dram_tensor("xt", [128, NH * L], bf16, kind="ExternalInput")
    xts_d = nc.dram_tensor("xts", [128, NH * IH], bf16, kind="ExternalInput")
    w1s_d = nc.dram_tensor("w1s", [NH, 128, NH * 128], bf16, kind="ExternalInput")
    w1e_d = nc.dram_tensor("w1e", [NH, 128, NH * 128], bf16, kind="ExternalInput")
    b1t_d = nc.dram_tensor("b1t", [128, NH], f32, kind="ExternalInput")
    w2t_d = nc.dram_tensor("w2t", [128, NH * NL], bf16, kind="ExternalInput")
    b2t_d = nc.dram_tensor("b2t", [128, GC], f32, kind="ExternalInput")
    # j-major output: per-partition (j) the (i, n) block is contiguous, so
    # the store DMA writes 1664B bursts instead of 52B (i-major was 8-13us
    # per store; this is <1us).  Host transposes back to (i, j, n).
    out_d = nc.dram_tensor("out", [L, IH, NL], f32, kind="ExternalOutput")

    with TileContext(nc) as tc:
        def body():
            with (
                tc.tile_pool(name="consts", bufs=1) as cp,
                tc.tile_pool(name="pp", bufs=2, space="PSUM") as pp,
                tc.tile_pool(name="sump", bufs=4) as sp,
                tc.tile_pool(name="gelp", bufs=5) as gp,
                tc.tile_pool(name="outp", bufs=3) as op,
                tc.tile_pool(name="w1p", bufs=1) as wp,
                tc.tile_pool(name="chainp", bufs=1) as chp,
            ):
                # Fused multi-chunk loads: one DMA each (HWDGE issue rate is
                # ~0.6us per dma_start, so small separate loads serialize the
                # startup critical path).
                XTf = cp.tile([128, NH * L], bf16, tag="xtf", name="XTf")
                hl = NH * L // 2
                # XT first (the xe path is the longer critical path:
                # pxe needs all of XT plus W1E0), one combined DMA.
                nc.sync.dma_start(out=XTf, in_=xt_d[:, :])
                W1E0 = wp.tile([128, NH * 128], bf16, tag="w1", bufs=10,
                               name="W1E0")
                nc.sync.dma_start(out=W1E0, in_=w1e_d[0])
                XTSf = cp.tile([128, NH * IH], bf16, tag="xtsf", name="XTSf")
                nc.sync.dma_start(out=XTSf, in_=xts_d[:, :])
                W1S0 = wp.tile([128, NH * 128], bf16, tag="w1", bufs=10,
                               name="W1S0")
                nc.sync.dma_start(out=W1S0, in_=w1s_d[0])
                XT = [XTf[:, h * L:(h + 1) * L] for h in range(NH)]
                XEf = cp.tile([128, NH * L], bf16, tag="xef", name="XEf")
                XSBf = cp.tile([128, NH * IH], f32, tag="xsbf", name="XSBf")
                XE = [XEf[:, h * L:(h + 1) * L] for h in range(NH)]
                XSB = [XSBf[:, h * IH:(h + 1) * IH] for h in range(NH)]

                # Projections, kc-tiled W1 loads: only the 12 [128,128] column
                # tiles of W1 needed for output chunk kc are loaded before its
                # matmuls, so the first gelu group starts ~0.8MB (not 4.7MB)
                # into the weight stream.  xeT over all L columns, xsT over
                # this core's IH columns, b1 folded into xs.  fp32 matmuls.
                def load_w1(k):
                    W1Ek = wp.tile([128, NH * 128], bf16, tag="w1", bufs=10,
                                   name=f"W1E{k}")
                    nc.sync.dma_start(out=W1Ek, in_=w1e_d[k])
                    W1Sk = wp.tile([128, NH * 128], bf16, tag="w1", bufs=10,
                                   name=f"W1S{k}")
                    nc.sync.dma_start(out=W1Sk, in_=w1s_d[k])
                    return W1Ek, W1Sk

                B1T = cp.tile([128, NH], f32, tag="b1t", name="B1T")
                nc.sync.dma_start(out=B1T, in_=b1t_d[:, :])
                w1_tiles = {0: (W1E0, W1S0)}
                W2Bf = cp.tile([128, NH * NL], bf16, tag="w2bf", name="W2Bf")
                nc.sync.dma_start(out=W2Bf, in_=w2t_d[:, :])
                XTS = [XTSf[:, h * IH:(h + 1) * IH] for h in range(NH)]
                W2B = [W2Bf[:, h * NL:(h + 1) * NL] for h in range(NH)]

                def proj(k):
                    if k not in w1_tiles:
                        w1_tiles[k] = load_w1(k)
                    W1Ek, W1Sk = w1_tiles[k]
                    pxe = pp.tile([128, L], f32, tag="pxe", name=f"pxe{k}")
                    for h in range(NH):
                        nc.tensor.matmul(
                            pxe,
                            lhsT=W1Ek[:, h * 128:(h + 1) * 128],
                            rhs=XT[h],
                            start=(h == 0),
                            stop=(h == NH - 1),
                        )
                    nc.vector.tensor_copy(out=XE[k], in_=pxe)
                    pxs = pp.tile([128, IH], f32, tag="pxs", name=f"pxs{k}")
                    for h in range(NH):
                        nc.tensor.matmul(
                            pxs,
                            lhsT=W1Sk[:, h * 128:(h + 1) * 128],
                            rhs=XTS[h],
                            start=(h == 0),
                            stop=(h == NH - 1),
                        )
                    nc.vector.tensor_scalar_add(
                        out=XSB[k], in0=pxs, scalar1=B1T[:, k:k + 1]
                    )

                # Only chunk 0's projection runs before the steady state;
                # proj(k+1) is interleaved after group 0's chunk-k bcasts so
                # the first broadcast block starts ~6us earlier.
                proj(0)
                ONE1 = cp.tile([1, 128], f32, tag="one1", name="ONE1")
                nc.vector.memset(ONE1, 1.0)
                B2T = cp.tile([128, GC], f32, tag="b2t", name="B2T")
                nc.sync.dma_start(out=B2T, in_=b2t_d[:, :])
                for k in range(1, NH):
                    w1_tiles[k] = load_w1(k)

                # Steady state over i-groups.
                for g in range(NGRP):
                    gel = []
                    chain_ops = []
                    for c in range(NH):
                        dcols = DVE_COLS.get(g, 0) if c == CHAIN_C else 0
                        # Tail chain: while DVE idles at the end, it takes a
                        # slice of the very last tile so ACT finishes sooner.
                        tail_chain = False
                        # The chain tile's input must outlive the whole
                        # group (its last op runs ~5 chunks later), so it
                        # gets its own tag outside the "sum" rotation.
                        st = sp.tile(
                            [128, G * L], bf16,
                            tag="sumc" if dcols else "sum",
                            bufs=1 if dcols else None,
                            name=f"sum{g}_{c}",
                        )
                        for il in range(G):
                            i = g * G + il
                            nc.vector.tensor_scalar_add(
                                out=st[:, il * L:(il + 1) * L],
                                in0=XE[c],
                                scalar1=XSB[c][:, i:i + 1],
                            )
                        if g == 0 and c + 1 < NH:
                            proj(c + 1)
                        # Split tiles get their own tag: the DVE chain writes
                        # them early; sharing the "gel" rotation would stall
                        # the chain on PE's consumption of an earlier buffer.
                        gt = gp.tile(
                            [128, G * L], bf16,
                            tag="gelc" if (dcols and not tail_chain) else "gel",
                            bufs=1 if (dcols and not tail_chain) else None,
                            name=f"gel{g}_{c}",
                        )
                        if dcols:
                            # ACT takes the HEAD cols (ready after the first
                            # few broadcast-adds -> early start), the DVE
                            # chain takes the TAIL cols.
                            rem = G * L - dcols
                            nsp = 2 if g == 0 else 1
                            hr = rem // nsp
                            for x in range(nsp if rem else 0):
                                cs = slice(x * hr, (x + 1) * hr)
                                nc.scalar.activation(
                                    out=gt[:, cs], in_=st[:, cs], func=GELU
                                )
                            # DVE clipped-cubic gelu on the tail dcols:
                            # 5 ops, queued one per chunk below.
                            dmax = max(DVE_COLS.values())
                            zt = chp.tile([128, dmax], bf16, tag="z",
                                          name=f"z{g}_{c}")[:, :dcols]
                            z2t = chp.tile([128, dmax], bf16, tag="z2",
                                           name=f"z2{g}_{c}")[:, :dcols]
                            ops_list = [
                                lambda zt=zt, st=st, rem=rem:
                                nc.vector.tensor_scalar(
                                    out=zt, in0=st[:, rem:], scalar1=-CLIP,
                                    scalar2=CLIP, op0=ALU.max, op1=ALU.min,
                                ),
                                lambda zt=zt, z2t=z2t: nc.vector.tensor_tensor(
                                    out=z2t, in0=zt, in1=zt, op=ALU.mult
                                ),
                                # q overwrites z2 and p overwrites z in place
                                # (elementwise, stream-aligned: safe on DVE).
                                lambda z2t=z2t: nc.vector.tensor_scalar(
                                    out=z2t, in0=z2t, scalar1=-BETA,
                                    scalar2=ALPHA, op0=ALU.mult, op1=ALU.add,
                                ),
                                lambda zt=zt, z2t=z2t: nc.vector.tensor_tensor(
                                    out=zt, in0=z2t, in1=zt, op=ALU.mult
                                ),
                                # stt is 1x on HW; ts(+0.5) then tt(mult)
                                # stays at 4x/2x.
                                lambda zt=zt, z2t=z2t:
                                nc.vector.tensor_scalar_add(
                                    out=z2t, in0=zt, scalar1=0.5,
                                ),
                                lambda gt=gt, z2t=z2t, st=st, rem=rem:
                                nc.vector.tensor_tensor(
                                    out=gt[:, rem:], in0=z2t,
                                    in1=st[:, rem:], op=ALU.mult,
                                ),
                            ]
                            if tail_chain:
                                chain_ops2 = ops_list
                            else:
                                chain_ops = ops_list
                        elif g == NGRP - 1 and c == NH - 1:
                            # Quarter the final activation (tail shrink).
                            q = G * L // 4
                            for x in range(4):
                                nc.scalar.activation(
                                    out=gt[:, x * q:(x + 1) * q],
                                    in_=st[:, x * q:(x + 1) * q],
                                    func=GELU,
                                )
                        elif g == 0 and c == 1:
                            half = G * L // 2
                            nc.scalar.activation(
                                out=gt[:, :half], in_=st[:, :half], func=GELU
                            )
                            nc.scalar.activation(
                                out=gt[:, half:], in_=st[:, half:], func=GELU
                            )
                        elif g == 0 and c == 0:
                            # Split the first activation so ACT starts after
                            # the first 8 broadcast-adds (startup shrink).
                            q = G * L // 4
                            for x in range(4):
                                nc.scalar.activation(
                                    out=gt[:, x * q:(x + 1) * q],
                                    in_=st[:, x * q:(x + 1) * q],
                                    func=GELU,
                                )
                        else:
                            nc.scalar.activation(out=gt, in_=st, func=GELU)
                        gel.append(gt)
                        if chain_ops:
                            chain_ops.pop(0)()
                        if tail_chain:
                            # DVE is drained at this point; run the 6 chain
                            # ops back-to-back.
                            for op_ in chain_ops2:
                                op_()
                    ps = [
                        pp.tile([128, GC], f32, tag=f"ps{jt}", name=f"ps{g}_{jt}")
                        for jt in range(2)
                    ]
                    # PSUM has_written clears at BANK granularity on start=True,
                    # so exactly one start per psum tile: a rank-1 preload
                    # (ones-column x b2-row) seeds the bank with b2, turning
                    # the post-accumulation DVE add into a cheaper copy.
                    for jt in range(2):
                        nc.tensor.matmul(
                            ps[jt],
                            lhsT=ONE1,
                            rhs=B2T[0:1, :],
                            start=True,
                            stop=False,
                            skip_group_check=True,
                        )
                    corder = list(range(NH))
                    for ci, c in enumerate(corder):
                        for il in range(G):
                            for jt in range(2):
                                nc.tensor.matmul(
                                    ps[jt][:, il * NL:(il + 1) * NL],
                                    lhsT=gel[c][:, il * L + jt * 128: il * L + jt * 128 + 128],
                                    rhs=W2B[c],
                                    start=False,
                                    stop=(ci == NH - 1 and il == G - 1),
                                    skip_group_check=True,
                                )
                    nhalf = 1
                    hw_ = G // nhalf
                    for jt in range(2):
                        ob = op.tile([128, GC], f32, tag="ob", name=f"ob{g}_{jt}")
                        for x in range(nhalf):
                            cs = slice(x * hw_ * NL, (x + 1) * hw_ * NL)
                            nc.vector.tensor_copy(
                                out=ob[:, cs], in_=ps[jt][:, cs]
                            )
                            ov = out_d[
                                jt * 128:(jt + 1) * 128,
                                g * G + x * hw_:g * G + (x + 1) * hw_, :,
                            ]
                            nc.sync.dma_start(
                                out=ov,
                                in_=ob[:, cs].rearrange("p (i n) -> p i n", n=NL),
                            )

        if repeat == 1:
            body()
        else:
            with tc.For_i(0, repeat, 1):
                body()

    nc.compile()
    return nc


def _get_program(repeat=1):
    if repeat not in _CACHE:
        _CACHE[repeat] = _build(repeat)
    return _CACHE[repeat]


def make_in_maps(hidden_states, W1, b1, W2, b2):
    hidden_states = np.asarray(hidden_states, dtype=np.float32)
    W1 = np.asarray(W1, dtype=np.float32)
    b1 = np.asarray(b1, dtype=np.float32)
    W2 = np.asarray(W2, dtype=np.float32)
    b2 = np.asarray(b2, dtype=np.float32)

    import ml_dtypes

    def w1_prep(w):
        # [(c p), (k kk)] -> [k, p, (c kk)]: per-kc slab, direct tile layout.
        return np.ascontiguousarray(
            w.reshape(NH, 128, NH, 128).transpose(2, 1, 0, 3).reshape(NH, 128, NH * 128)
        ).astype(ml_dtypes.bfloat16)

    w1s = w1_prep(W1[:H])
    w1e = w1_prep(W1[H:])
    w2t = np.ascontiguousarray(
        W2.reshape(NH, 128, NL).transpose(1, 0, 2).reshape(128, NH * NL)
    ).astype(ml_dtypes.bfloat16)
    b1t = np.ascontiguousarray(b1.reshape(NH, 128).T)
    b2t = np.ascontiguousarray(np.tile(b2, (128, G)))

    in_maps = []
    for core in range(8):
        b, ih = core // 2, core % 2
        # [s, (c p)] -> [p, (c s)]: direct tile layouts.
        xt = np.ascontiguousarray(
            hidden_states[b].reshape(L, NH, 128).transpose(2, 1, 0).reshape(128, NH * L)
        ).astype(ml_dtypes.bfloat16)
        xts = np.ascontiguousarray(
            hidden_states[b][ih * IH:(ih + 1) * IH]
            .reshape(IH, NH, 128).transpose(2, 1, 0).reshape(128, NH * IH)
        ).astype(ml_dtypes.bfloat16)
        in_maps.append(
            {
                "xt": xt,
                "xts": xts,
                "w1s": w1s,
                "w1e": w1e,
                "b1t": b1t,
                "w2t": w2t,
                "b2t": b2t,
            }
        )
    return in_maps


def kernel(hidden_states, W1, b1, W2, b2):
    from concourse.bass_utils import run_bass_kernel_spmd

    nc = _get_program()
    in_maps = make_in_maps(hidden_states, W1, b1, W2, b2)
    res = run_bass_kernel_spmd(nc, in_maps, core_ids=list(range(8)))

    out = np.empty((B, L, L, NL), dtype=np.float32)
    for core in range(8):
        b, ih = core // 2, core % 2
        out[b, ih * IH:(ih + 1) * IH] = res.results[core]["out"].transpose(1, 0, 2)
    return out

